# revision 13
# baseline (speedup 1.0000x reference)
"""Trainium2 Bass kernel for single-head cross-modal attention.

Problem: B=8, S=2048, D=1024 (fp32 inputs)
    q = image_emb @ Wq.T + bq
    k = text_emb  @ Wk.T + bk
    v = text_emb  @ Wv.T + bv
    out = softmax(q @ k.T / sqrt(D)) @ v

Sharding: data-parallel over batch — core b handles batch element b.

Key algebraic trick: scores = (Xi Wq^T + bq)(Xt Wk^T + bk)^T.  The
per-q-row bias terms cancel in softmax, so
    softmax(scores) = softmax(Xi M Xt^T + 1 c^T),   M  = Wq^T Wk
                                                    c  = Xt (Wk^T bq)
M and c are precomputed on host (fp32), so the on-chip Q projection
disappears entirely: K~^T = M Xt^T takes the place of both projections
on the scores path (saves 2.1e9 MACs/core, ~14%).

Per-core dataflow (all matmuls bf16 with fp32 PSUM accumulation):
  - X^T prepared on host (bf16 cast + transpose), streamed in 512-col
    chunks; each chunk feeds BOTH the K~ projection and the V
    projection (XtT is DMA'd once).
  - K~T in [e, s] layout, V in natural [s, e] layout.
  - scores^T [k_part, q_free] = K~T_tile.T @ XiT, so exp(scores)^T is
    directly the stationary operand of the P@V matmul: the 2048x2048
    probability matrix is never transposed on chip.
  - softmax without max-subtraction (scores ~ N(0,1), |s| <= ~6: exp is
    safe in fp32).  The c bias rides the exp activation's per-partition
    bias operand.  Row sums come from N=1 matmuls against a ones
    column (sharing the stationary operand with the P@V matmuls).
  - final normalize fused: out = (att_psum * recip) + bv_bcast in one
    DVE op per 512-wide chunk; output DMA split across sync+scalar
    rings.
  - a short warmup matmul stream on memset data bridges the initial
    DMA wait and lifts the PE HAM clock gate before real work arrives
    (the warmup accumulates zeros into what becomes the bv broadcast,
    so it cannot be dead-code-eliminated).
"""

import sys
import os

for _p in ("/opt/trn_rl_repo", "/root/.axon_site/_ro/trn_rl_repo"):
    if os.path.isdir(_p) and _p not in sys.path:
        sys.path.insert(0, _p)

import numpy as np
import ml_dtypes

import concourse.bass as bass
import concourse.mybir as mybir
import concourse.tile as tile
from concourse import bacc
from concourse.bass_utils import run_bass_kernel_spmd

BF16 = mybir.dt.bfloat16
F32 = mybir.dt.float32
AF = mybir.ActivationFunctionType
ALU = mybir.AluOpType

B, S, D = 8, 2048, 1024
P = 128
ND = D // P          # 8  d/e tiles
NS = S // P          # 16 s tiles
QC = 512             # chunk width (matmul free dim / PSUM bank)
NQC = S // QC        # 4
EC = 512             # e chunk width for V / output
SCALE = 1.0 / float(np.sqrt(D))
NWARM = 24           # warmup matmuls (HAM + DMA-wait bridge)
WC = 256             # warmup matmul free dim

_CACHE = {}


def _build_nc():
    nc = bacc.Bacc("TRN2", target_bir_lowering=False, debug=False, num_devices=8)

    xi_d = nc.dram_tensor("xiT", [D, S], BF16, kind="ExternalInput").ap()
    xt_d = nc.dram_tensor("xtT", [D, S], BF16, kind="ExternalInput").ap()
    mt_d = nc.dram_tensor("mt", [D, D], BF16, kind="ExternalInput").ap()   # Wk.T@Wq
    wvt_d = nc.dram_tensor("wvt", [D, D], BF16, kind="ExternalInput").ap()  # Wv.T
    cv_d = nc.dram_tensor("cv", [P, NS], F32, kind="ExternalInput").ap()
    bvb_d = nc.dram_tensor("bvb", [P, D], F32, kind="ExternalInput").ap()
    out_d = nc.dram_tensor("out", [S, D], F32, kind="ExternalOutput").ap()

    with tile.TileContext(nc) as tc:
        _emit(nc, tc, xi_d, xt_d, mt_d, wvt_d, cv_d, bvb_d, out_d)
    nc.compile()
    return nc


def _emit(nc, tc, xi_d, xt_d, mt_d, wvt_d, cv_d, bvb_d, out_d):
    NH = QC // P  # 4 q_tiles per chunk
    with (
        tc.tile_pool(name="const", bufs=1) as pc,
        tc.tile_pool(name="qkv", bufs=1) as pqkv,
        tc.tile_pool(name="psST", bufs=2, space="PSUM") as psST,
    ):
        # persistent activations
        kt = pqkv.tile([P, ND, S], BF16, name="kt", tag="kt")    # K~T[e,k]
        v = pqkv.tile([P, NS, D], BF16, name="v", tag="v")       # V[s,e]
        xi = pqkv.tile([P, ND, S], BF16, name="xi", tag="xi")    # XiT[e,q]

        # constants
        cv = pc.tile([P, NS], F32, name="cv", tag="cv")          # c per k-tile
        wl = pc.tile([1, P], BF16, name="wl", tag="wl")          # warmup lhsT
        wr = pc.tile([1, WC], BF16, name="wr", tag="wr")         # warmup rhs
        bv_bcast = pc.tile([P, D], F32, name="bv_bcast", tag="bv_bcast")
        ones_col = pc.tile([P, 1], BF16, name="ones_col", tag="ones_col")

        with (
            tc.tile_pool(name="w", bufs=1) as pw,
            tc.tile_pool(name="xs", bufs=3) as pxs,
            tc.tile_pool(name="psP", bufs=6, space="PSUM") as psP,
        ):
            mt_sb = pw.tile([P, ND, D], BF16, name="mt_sb", tag="mt_sb")
            wv_sb = pw.tile([P, ND, D], BF16, name="wv_sb", tag="wv_sb")

            nc.scalar.dma_start(cv[:], cv_d[:])
            nc.vector.memset(wl[:], 1.0)
            nc.vector.memset(wr[:], 1.0 / NWARM)

            # Consolidated transfers: one big DMA engages all 16 SDMA
            # engines (~341 GB/s) vs ~150-200 for a string of small ones.
            # The first-needed tensors (mt, xc0) are split across both
            # HWDGE rings so their halves land in parallel.
            HD = ND // 2
            mt_r = mt_d.rearrange("(nd p) e -> p nd e", p=P)
            wv_r = wvt_d.rearrange("(nd p) e -> p nd e", p=P)
            xi_r = xi_d.rearrange("(nd p) s -> p nd s", p=P)
            xt_r = xt_d.rearrange("(nd p) s -> p nd s", p=P)
            nc.sync.dma_start(mt_sb[:, 0:HD, :], mt_r[:, 0:HD, :])
            nc.scalar.dma_start(mt_sb[:, HD:ND, :], mt_r[:, HD:ND, :])
            nc.gpsimd.dma_start(wv_sb[:], wv_r[:])
            nc.gpsimd.dma_start(xi[:], xi_r[:])
            nc.gpsimd.dma_start(bv_bcast[:], bvb_d[:])

            # --- warmup: bf16 matmuls accumulating 1/NWARM into what
            # becomes ones_col (live data, so not DCE-able).  Keeps PE
            # busy during the initial DMA wait and warms the HAM gate.
            ps_w = psP.tile([P, WC], F32, name="ps_w", tag="ps")
            for i in range(NWARM):
                nc.tensor.matmul(ps_w[:], wl[:], wr[:],
                                 start=(i == 0), stop=(i == NWARM - 1))
            nc.vector.tensor_copy(ones_col[:], ps_w[:, 0:1])

            # --- projections: stream XtT chunks once; K~ then V per chunk.
            # Order K0 K1 V0 K2 V1 K3 V2 V3 gives the wv DMA time to land.
            xcs = {}

            def load_chunk(c):
                xc = pxs.tile([P, ND, QC], BF16, name="xc", tag="xs")
                cs = slice(c * QC, (c + 1) * QC)
                if c <= 1:
                    nc.sync.dma_start(xc[:, 0:HD, :], xt_r[:, 0:HD, cs])
                    nc.scalar.dma_start(xc[:, HD:ND, :], xt_r[:, HD:ND, cs])
                else:
                    nc.gpsimd.dma_start(xc[:], xt_r[:, :, cs])
                xcs[c] = xc

            def kproj(c):
                xc = xcs[c]
                for et in range(ND):
                    ps = psP.tile([P, QC], F32, name="ps", tag="ps")
                    for d in range(ND):
                        nc.tensor.matmul(
                            ps[:], mt_sb[:, d, et * P:(et + 1) * P],
                            xc[:, d, :],
                            start=(d == 0), stop=(d == ND - 1))
                    nc.vector.tensor_copy(
                        kt[:, et, c * QC:(c + 1) * QC], ps[:])

            def vproj(c):
                xc = xcs[c]
                for si in range(NH):
                    st = c * NH + si
                    ps0 = psP.tile([P, EC], F32, name="ps0", tag="ps")
                    ps1 = psP.tile([P, EC], F32, name="ps1", tag="ps")
                    for d in range(ND):
                        lhs = xc[:, d, si * P:(si + 1) * P]
                        nc.tensor.matmul(ps0[:], lhs, wv_sb[:, d, 0:EC],
                                         start=(d == 0), stop=(d == ND - 1))
                        nc.tensor.matmul(ps1[:], lhs, wv_sb[:, d, EC:D],
                                         start=(d == 0), stop=(d == ND - 1))
                    nc.vector.tensor_copy(v[:, st, 0:EC], ps0[:])
                    nc.vector.tensor_copy(v[:, st, EC:D], ps1[:])

            load_chunk(0)
            load_chunk(1)
            kproj(0)
            load_chunk(2)
            kproj(1)
            vproj(0)
            load_chunk(3)
            kproj(2)
            vproj(1)
            kproj(3)
            vproj(2)
            vproj(3)

        # --- attention ---
        with (
            tc.tile_pool(name="et", bufs=3) as pet,
            tc.tile_pool(name="outp", bufs=2) as pout,
            tc.tile_pool(name="stat", bufs=4) as pstat,
            tc.tile_pool(name="psAV", bufs=4, space="PSUM") as psAV,
            tc.tile_pool(name="psRS", bufs=2, space="PSUM") as psRS,
        ):
            for qc in range(NQC):
                # scores^T for this q chunk: ET[kk, q] = exp(scale*K~T.T@XiT + c)
                et_t = pet.tile([P, NS, QC], BF16, name="et_t", tag="et")
                for kk in range(NS):
                    st_ps = psST.tile([P, QC], F32, name="st_ps", tag="st")
                    for e in range(ND):
                        nc.tensor.matmul(
                            st_ps[:],
                            kt[:, e, kk * P:(kk + 1) * P],
                            xi[:, e, qc * QC:(qc + 1) * QC],
                            start=(e == 0), stop=(e == ND - 1))
                    nc.scalar.activation(et_t[:, kk, :], st_ps[:], AF.Exp,
                                         bias=cv[:, kk:kk + 1], scale=SCALE)

                # attended[q_t, :] = (ET.T @ V) * recip + bv
                for qs in range(NH):
                    a0 = psAV.tile([P, EC], F32, name="a0", tag="av")
                    a1 = psAV.tile([P, EC], F32, name="a1", tag="av")
                    rs = psRS.tile([P, 1], F32, name="rs", tag="rs")
                    for kk in range(NS):
                        lhs = et_t[:, kk, qs * P:(qs + 1) * P]
                        nc.tensor.matmul(rs[:], lhs, ones_col[:],
                                         start=(kk == 0), stop=(kk == NS - 1))
                        nc.tensor.matmul(a0[:], lhs, v[:, kk, 0:EC],
                                         start=(kk == 0), stop=(kk == NS - 1))
                        nc.tensor.matmul(a1[:], lhs, v[:, kk, EC:D],
                                         start=(kk == 0), stop=(kk == NS - 1))
                    recip = pstat.tile([P, 1], F32, name="recip", tag="recip")
                    nc.vector.reciprocal(recip[:], rs[:])
                    ob = pout.tile([P, D], F32, name="ob", tag="ob")
                    q_tile = qc * NH + qs
                    nc.vector.scalar_tensor_tensor(
                        ob[:, 0:EC], a0[:], recip[:], bv_bcast[:, 0:EC],
                        op0=ALU.mult, op1=ALU.add)
                    nc.sync.dma_start(
                        out_d[q_tile * P:(q_tile + 1) * P, 0:EC], ob[:, 0:EC])
                    nc.vector.scalar_tensor_tensor(
                        ob[:, EC:D], a1[:], recip[:], bv_bcast[:, EC:D],
                        op0=ALU.mult, op1=ALU.add)
                    nc.scalar.dma_start(
                        out_d[q_tile * P:(q_tile + 1) * P, EC:D], ob[:, EC:D])


def get_nc():
    if "nc" not in _CACHE:
        _CACHE["nc"] = _build_nc()
    return _CACHE["nc"]


def _prep_inputs(image_emb, text_emb, Wq, bq, Wk, bk, Wv, bv):
    bf = ml_dtypes.bfloat16
    xi = np.asarray(image_emb).astype(bf)   # [B, S, D]
    xt = np.asarray(text_emb).astype(bf)
    xiT = np.ascontiguousarray(xi.transpose(0, 2, 1))  # [B, D, S]
    xtT = np.ascontiguousarray(xt.transpose(0, 2, 1))
    Wq32 = np.asarray(Wq, dtype=np.float32)
    Wk32 = np.asarray(Wk, dtype=np.float32)
    mt = np.ascontiguousarray(Wk32.T @ Wq32).astype(bf)       # M^T = Wk^T Wq
    wvt = np.ascontiguousarray(np.asarray(Wv).T).astype(bf)
    bq32 = np.asarray(bq, dtype=np.float32)
    # per-k score bias that survives softmax: c = Xt @ (Wk^T bq)
    u = Wk32.T @ bq32                                          # [D]
    c = np.asarray(text_emb, dtype=np.float32) @ u             # [B, S]
    cv = np.ascontiguousarray(c.reshape(B, NS, P).transpose(0, 2, 1))
    bvb = np.ascontiguousarray(
        np.broadcast_to(np.asarray(bv, dtype=np.float32), (P, D)))
    in_maps = []
    for b in range(B):
        in_maps.append({
            "xiT": xiT[b], "xtT": xtT[b],
            "mt": mt, "wvt": wvt,
            "cv": cv[b], "bvb": bvb,
        })
    return in_maps


def run(image_emb, text_emb, Wq, bq, Wk, bk, Wv, bv, trace=False, **spmd_kwargs):
    nc = get_nc()
    in_maps = _prep_inputs(image_emb, text_emb, Wq, bq, Wk, bk, Wv, bv)
    res = run_bass_kernel_spmd(nc, in_maps, list(range(B)), trace=trace,
                               **spmd_kwargs)
    out = np.stack([res.results[b]["out"] for b in range(B)], axis=0)
    return out, res


def kernel(image_emb, text_emb, edge_index=None, Wq=None, bq=None, Wk=None,
           bk=None, Wv=None, bv=None, **_unused):
    out, _ = run(image_emb, text_emb, Wq, bq, Wk, bk, Wv, bv, trace=False)
    return out


# revision 18
# speedup vs baseline: 1.0162x; 1.0162x over previous
"""Trainium2 Bass kernel for single-head cross-modal attention.

Problem: B=8, S=2048, D=1024 (fp32 inputs)
    q = image_emb @ Wq.T + bq
    k = text_emb  @ Wk.T + bk
    v = text_emb  @ Wv.T + bv
    out = softmax(q @ k.T / sqrt(D)) @ v

Sharding: data-parallel over batch — core b handles batch element b.

Key algebraic trick: scores = (Xi Wq^T + bq)(Xt Wk^T + bk)^T.  The
per-q-row bias terms cancel in softmax, so
    softmax(scores) = softmax(Xi M Xt^T + 1 c^T),   M  = Wq^T Wk
                                                    c  = Xt (Wk^T bq)
M and c are precomputed on host (fp32), so the on-chip Q projection
disappears entirely: K~^T = M Xt^T takes the place of both projections
on the scores path (saves 2.1e9 MACs/core, ~14%).

Per-core dataflow (all matmuls bf16 with fp32 PSUM accumulation):
  - X^T prepared on host (bf16 cast + transpose), streamed in 512-col
    chunks; each chunk feeds BOTH the K~ projection and the V
    projection (XtT is DMA'd once).
  - K~T in [e, s] layout, V in natural [s, e] layout.
  - scores^T [k_part, q_free] = K~T_tile.T @ XiT, so exp(scores)^T is
    directly the stationary operand of the P@V matmul: the 2048x2048
    probability matrix is never transposed on chip.
  - softmax without max-subtraction (scores ~ N(0,1), |s| <= ~6: exp is
    safe in fp32).  The c bias rides the exp activation's per-partition
    bias operand.  Row sums come from N=1 matmuls against a ones
    column (sharing the stationary operand with the P@V matmuls).
  - final normalize fused: out = (att_psum * recip) + bv_bcast in one
    DVE op per 512-wide chunk; output DMA split across sync+scalar
    rings.
  - a short warmup matmul stream on memset data bridges the initial
    DMA wait and lifts the PE HAM clock gate before real work arrives
    (the warmup accumulates zeros into what becomes the bv broadcast,
    so it cannot be dead-code-eliminated).
"""

import sys
import os

for _p in ("/opt/trn_rl_repo", "/root/.axon_site/_ro/trn_rl_repo"):
    if os.path.isdir(_p) and _p not in sys.path:
        sys.path.insert(0, _p)

import numpy as np
import ml_dtypes

import concourse.bass as bass
import concourse.mybir as mybir
import concourse.tile as tile
from concourse import bacc
from concourse.bass_utils import run_bass_kernel_spmd

BF16 = mybir.dt.bfloat16
F32 = mybir.dt.float32
AF = mybir.ActivationFunctionType
ALU = mybir.AluOpType

B, S, D = 8, 2048, 1024
P = 128
ND = D // P          # 8  d/e tiles
NS = S // P          # 16 s tiles
QC = 512             # chunk width (matmul free dim / PSUM bank)
NQC = S // QC        # 4
EC = 512             # e chunk width for V / output
SCALE = 1.0 / float(np.sqrt(D))
NWARM = 16           # warmup matmuls (HAM + DMA-wait bridge)
WC = 256             # warmup matmul free dim

_CACHE = {}


def _build_nc():
    nc = bacc.Bacc("TRN2", target_bir_lowering=False, debug=False, num_devices=8)

    xi_d = nc.dram_tensor("xiT", [D, S], BF16, kind="ExternalInput").ap()
    xt_d = nc.dram_tensor("xtT", [D, S], BF16, kind="ExternalInput").ap()
    mt_d = nc.dram_tensor("mt", [D, D], BF16, kind="ExternalInput").ap()   # Wk.T@Wq
    wvt_d = nc.dram_tensor("wvt", [D, D], BF16, kind="ExternalInput").ap()  # Wv.T
    cv_d = nc.dram_tensor("cv", [P, NS], F32, kind="ExternalInput").ap()
    bvb_d = nc.dram_tensor("bvb", [P, D], F32, kind="ExternalInput").ap()
    out_d = nc.dram_tensor("out", [S, D], F32, kind="ExternalOutput").ap()

    with tile.TileContext(nc) as tc:
        _emit(nc, tc, xi_d, xt_d, mt_d, wvt_d, cv_d, bvb_d, out_d)
    nc.compile()
    return nc


def _emit(nc, tc, xi_d, xt_d, mt_d, wvt_d, cv_d, bvb_d, out_d):
    NH = QC // P  # 4 q_tiles per chunk
    with (
        tc.tile_pool(name="const", bufs=1) as pc,
        tc.tile_pool(name="qkv", bufs=1) as pqkv,
        tc.tile_pool(name="psST", bufs=2, space="PSUM") as psST,
    ):
        # persistent activations
        kt = pqkv.tile([P, ND, S], BF16, name="kt", tag="kt")    # K~T[e,k]
        v = pqkv.tile([P, NS, D], BF16, name="v", tag="v")       # V[s,e]
        xi = pqkv.tile([P, ND, S], BF16, name="xi", tag="xi")    # XiT[e,q]

        # constants
        cv = pc.tile([P, NS], F32, name="cv", tag="cv")          # c per k-tile
        wl = pc.tile([1, P], BF16, name="wl", tag="wl")          # warmup lhsT
        wr = pc.tile([1, WC], BF16, name="wr", tag="wr")         # warmup rhs
        bv_bcast = pc.tile([P, D], F32, name="bv_bcast", tag="bv_bcast")
        ones_col = pc.tile([P, 1], BF16, name="ones_col", tag="ones_col")

        with (
            tc.tile_pool(name="w", bufs=1) as pw,
            tc.tile_pool(name="xs", bufs=3) as pxs,
            tc.tile_pool(name="psP", bufs=6, space="PSUM") as psP,
        ):
            mt_sb = pw.tile([P, ND, D], BF16, name="mt_sb", tag="mt_sb")
            wv_sb = pw.tile([P, ND, D], BF16, name="wv_sb", tag="wv_sb")

            nc.scalar.dma_start(cv[:], cv_d[:])
            nc.vector.memset(wl[:], 1.0)
            nc.vector.memset(wr[:], 1.0 / NWARM)

            # mt is needed first: stream its d-slices in parallel on both
            # HWDGE rings (plain 2D transfers — HWDGE is slow on 3D APs).
            # Everything else rides SWDGE (gpsimd), which handles big 3D
            # transfers at full rate; xc0 goes first there.
            HD = ND // 2
            wv_r = wvt_d.rearrange("(nd p) e -> p nd e", p=P)
            xi_r = xi_d.rearrange("(nd p) s -> p nd s", p=P)
            xt_r = xt_d.rearrange("(nd p) s -> p nd s", p=P)
            for d in range(HD):
                nc.sync.dma_start(mt_sb[:, d, :], mt_d[d * P:(d + 1) * P, :])
                nc.scalar.dma_start(mt_sb[:, HD + d, :],
                                    mt_d[(HD + d) * P:(HD + d + 1) * P, :])

            # --- warmup: bf16 matmuls accumulating 1/NWARM into what
            # becomes ones_col (live data, so not DCE-able).  Keeps PE
            # busy during the initial DMA wait and warms the HAM gate.
            ps_w = psP.tile([P, WC], F32, name="ps_w", tag="ps")
            for i in range(NWARM):
                nc.tensor.matmul(ps_w[:], wl[:], wr[:],
                                 start=(i == 0), stop=(i == NWARM - 1))
            nc.vector.tensor_copy(ones_col[:], ps_w[:, 0:1])

            # --- projections: stream XtT chunks once; K~ then V per chunk.
            # Order K0 K1 V0 K2 V1 K3 V2 V3 gives the wv DMA time to land.
            xcs = {}

            def load_chunk(c, hwdge=False):
                xc = pxs.tile([P, ND, QC], BF16, name="xc", tag="xs")
                cs = slice(c * QC, (c + 1) * QC)
                if hwdge:
                    for d in range(ND):
                        eng = nc.sync if d % 2 == 0 else nc.scalar
                        eng.dma_start(
                            xc[:, d, :],
                            xt_d[d * P:(d + 1) * P, c * QC:(c + 1) * QC])
                else:
                    nc.gpsimd.dma_start(xc[:], xt_r[:, :, cs])
                xcs[c] = xc

            load_chunk(0)
            load_chunk(1, hwdge=True)
            nc.gpsimd.dma_start(wv_sb[:], wv_r[:])
            nc.gpsimd.dma_start(xi[:], xi_r[:])
            nc.gpsimd.dma_start(bv_bcast[:], bvb_d[:])

            def kproj(c):
                xc = xcs[c]
                for et in range(ND):
                    ps = psP.tile([P, QC], F32, name="ps", tag="ps")
                    for d in range(ND):
                        nc.tensor.matmul(
                            ps[:], mt_sb[:, d, et * P:(et + 1) * P],
                            xc[:, d, :],
                            start=(d == 0), stop=(d == ND - 1))
                    nc.vector.tensor_copy(
                        kt[:, et, c * QC:(c + 1) * QC], ps[:])

            def vproj(c):
                xc = xcs[c]
                for si in range(NH):
                    st = c * NH + si
                    ps0 = psP.tile([P, EC], F32, name="ps0", tag="ps")
                    ps1 = psP.tile([P, EC], F32, name="ps1", tag="ps")
                    for d in range(ND):
                        lhs = xc[:, d, si * P:(si + 1) * P]
                        nc.tensor.matmul(ps0[:], lhs, wv_sb[:, d, 0:EC],
                                         start=(d == 0), stop=(d == ND - 1))
                        nc.tensor.matmul(ps1[:], lhs, wv_sb[:, d, EC:D],
                                         start=(d == 0), stop=(d == ND - 1))
                    nc.vector.tensor_copy(v[:, st, 0:EC], ps0[:])
                    nc.vector.tensor_copy(v[:, st, EC:D], ps1[:])

            kproj(0)
            load_chunk(2)
            kproj(1)
            vproj(0)
            load_chunk(3)
            kproj(2)
            vproj(1)
            kproj(3)
            vproj(2)
            vproj(3)

        # --- attention ---
        with (
            tc.tile_pool(name="et", bufs=3) as pet,
            tc.tile_pool(name="outp", bufs=2) as pout,
            tc.tile_pool(name="stat", bufs=4) as pstat,
            tc.tile_pool(name="psAV", bufs=4, space="PSUM") as psAV,
            tc.tile_pool(name="psRS", bufs=2, space="PSUM") as psRS,
        ):
            for qc in range(NQC):
                # scores^T for this q chunk: ET[kk, q] = exp(scale*K~T.T@XiT + c)
                et_t = pet.tile([P, NS, QC], BF16, name="et_t", tag="et")
                for kk in range(NS):
                    st_ps = psST.tile([P, QC], F32, name="st_ps", tag="st")
                    for e in range(ND):
                        nc.tensor.matmul(
                            st_ps[:],
                            kt[:, e, kk * P:(kk + 1) * P],
                            xi[:, e, qc * QC:(qc + 1) * QC],
                            start=(e == 0), stop=(e == ND - 1))
                    nc.scalar.activation(et_t[:, kk, :], st_ps[:], AF.Exp,
                                         bias=cv[:, kk:kk + 1], scale=SCALE)

                # attended[q_t, :] = (ET.T @ V) * recip + bv
                for qs in range(NH):
                    a0 = psAV.tile([P, EC], F32, name="a0", tag="av")
                    a1 = psAV.tile([P, EC], F32, name="a1", tag="av")
                    rs = psRS.tile([P, 1], F32, name="rs", tag="rs")
                    for kk in range(NS):
                        lhs = et_t[:, kk, qs * P:(qs + 1) * P]
                        nc.tensor.matmul(rs[:], lhs, ones_col[:],
                                         start=(kk == 0), stop=(kk == NS - 1))
                        nc.tensor.matmul(a0[:], lhs, v[:, kk, 0:EC],
                                         start=(kk == 0), stop=(kk == NS - 1))
                        nc.tensor.matmul(a1[:], lhs, v[:, kk, EC:D],
                                         start=(kk == 0), stop=(kk == NS - 1))
                    recip = pstat.tile([P, 1], F32, name="recip", tag="recip")
                    nc.vector.reciprocal(recip[:], rs[:])
                    ob = pout.tile([P, D], F32, name="ob", tag="ob")
                    q_tile = qc * NH + qs
                    nc.vector.scalar_tensor_tensor(
                        ob[:, 0:EC], a0[:], recip[:], bv_bcast[:, 0:EC],
                        op0=ALU.mult, op1=ALU.add)
                    nc.sync.dma_start(
                        out_d[q_tile * P:(q_tile + 1) * P, 0:EC], ob[:, 0:EC])
                    nc.vector.scalar_tensor_tensor(
                        ob[:, EC:D], a1[:], recip[:], bv_bcast[:, EC:D],
                        op0=ALU.mult, op1=ALU.add)
                    nc.scalar.dma_start(
                        out_d[q_tile * P:(q_tile + 1) * P, EC:D], ob[:, EC:D])


def get_nc():
    if "nc" not in _CACHE:
        _CACHE["nc"] = _build_nc()
    return _CACHE["nc"]


def _prep_inputs(image_emb, text_emb, Wq, bq, Wk, bk, Wv, bv):
    bf = ml_dtypes.bfloat16
    xi = np.asarray(image_emb).astype(bf)   # [B, S, D]
    xt = np.asarray(text_emb).astype(bf)
    xiT = np.ascontiguousarray(xi.transpose(0, 2, 1))  # [B, D, S]
    xtT = np.ascontiguousarray(xt.transpose(0, 2, 1))
    Wq32 = np.asarray(Wq, dtype=np.float32)
    Wk32 = np.asarray(Wk, dtype=np.float32)
    mt = np.ascontiguousarray(Wk32.T @ Wq32).astype(bf)       # M^T = Wk^T Wq
    wvt = np.ascontiguousarray(np.asarray(Wv).T).astype(bf)
    bq32 = np.asarray(bq, dtype=np.float32)
    # per-k score bias that survives softmax: c = Xt @ (Wk^T bq)
    u = Wk32.T @ bq32                                          # [D]
    c = np.asarray(text_emb, dtype=np.float32) @ u             # [B, S]
    cv = np.ascontiguousarray(c.reshape(B, NS, P).transpose(0, 2, 1))
    bvb = np.ascontiguousarray(
        np.broadcast_to(np.asarray(bv, dtype=np.float32), (P, D)))
    in_maps = []
    for b in range(B):
        in_maps.append({
            "xiT": xiT[b], "xtT": xtT[b],
            "mt": mt, "wvt": wvt,
            "cv": cv[b], "bvb": bvb,
        })
    return in_maps


def run(image_emb, text_emb, Wq, bq, Wk, bk, Wv, bv, trace=False, **spmd_kwargs):
    nc = get_nc()
    in_maps = _prep_inputs(image_emb, text_emb, Wq, bq, Wk, bk, Wv, bv)
    res = run_bass_kernel_spmd(nc, in_maps, list(range(B)), trace=trace,
                               **spmd_kwargs)
    out = np.stack([res.results[b]["out"] for b in range(B)], axis=0)
    return out, res


def kernel(image_emb, text_emb, edge_index=None, Wq=None, bq=None, Wk=None,
           bk=None, Wv=None, bv=None, **_unused):
    out, _ = run(image_emb, text_emb, Wq, bq, Wk, bk, Wv, bv, trace=False)
    return out


# revision 22
# speedup vs baseline: 1.0213x; 1.0051x over previous
"""Trainium2 Bass kernel for single-head cross-modal attention.

Problem: B=8, S=2048, D=1024 (fp32 inputs)
    q = image_emb @ Wq.T + bq
    k = text_emb  @ Wk.T + bk
    v = text_emb  @ Wv.T + bv
    out = softmax(q @ k.T / sqrt(D)) @ v

Sharding: data-parallel over batch — core b handles batch element b.

Key algebraic trick: scores = (Xi Wq^T + bq)(Xt Wk^T + bk)^T.  The
per-q-row bias terms cancel in softmax, so
    softmax(scores) = softmax(Xi M Xt^T + 1 c^T),   M  = Wq^T Wk
                                                    c  = Xt (Wk^T bq)
M and c are precomputed on host (fp32), so the on-chip Q projection
disappears entirely: K~^T = M Xt^T takes the place of both projections
on the scores path (saves 2.1e9 MACs/core, ~14%).

Per-core dataflow (all matmuls bf16 with fp32 PSUM accumulation):
  - X^T prepared on host (bf16 cast + transpose), streamed in 512-col
    chunks; each chunk feeds BOTH the K~ projection and the V
    projection (XtT is DMA'd once).
  - K~T in [e, s] layout, V in natural [s, e] layout.
  - scores^T [k_part, q_free] = K~T_tile.T @ XiT, so exp(scores)^T is
    directly the stationary operand of the P@V matmul: the 2048x2048
    probability matrix is never transposed on chip.
  - softmax without max-subtraction (scores ~ N(0,1), |s| <= ~6: exp is
    safe in fp32).  The c bias rides the exp activation's per-partition
    bias operand.  Row sums come from N=1 matmuls against a ones
    column (sharing the stationary operand with the P@V matmuls).
  - final normalize fused: out = (att_psum * recip) + bv_bcast in one
    DVE op per 512-wide chunk; output DMA split across sync+scalar
    rings.
  - a short warmup matmul stream on memset data bridges the initial
    DMA wait and lifts the PE HAM clock gate before real work arrives
    (the warmup accumulates zeros into what becomes the bv broadcast,
    so it cannot be dead-code-eliminated).
"""

import sys
import os

for _p in ("/opt/trn_rl_repo", "/root/.axon_site/_ro/trn_rl_repo"):
    if os.path.isdir(_p) and _p not in sys.path:
        sys.path.insert(0, _p)

import numpy as np
import ml_dtypes

import concourse.bass as bass
import concourse.mybir as mybir
import concourse.tile as tile
from concourse import bacc
from concourse.bass_utils import run_bass_kernel_spmd

BF16 = mybir.dt.bfloat16
F32 = mybir.dt.float32
AF = mybir.ActivationFunctionType
ALU = mybir.AluOpType

B, S, D = 8, 2048, 1024
P = 128
ND = D // P          # 8  d/e tiles
NS = S // P          # 16 s tiles
QC = 512             # chunk width (matmul free dim / PSUM bank)
NQC = S // QC        # 4
EC = 512             # e chunk width for V / output
SCALE = 1.0 / float(np.sqrt(D))
NWARM = 16           # warmup matmuls (HAM + DMA-wait bridge)
WC = 256             # warmup matmul free dim

_CACHE = {}


def _build_nc():
    nc = bacc.Bacc("TRN2", target_bir_lowering=False, debug=False, num_devices=8)

    xi_d = nc.dram_tensor("xiT", [D, S], BF16, kind="ExternalInput").ap()
    xt_d = nc.dram_tensor("xtT", [D, S], BF16, kind="ExternalInput").ap()
    mt_d = nc.dram_tensor("mt", [D, D], BF16, kind="ExternalInput").ap()   # Wk.T@Wq
    wvt_d = nc.dram_tensor("wvt", [D, D], BF16, kind="ExternalInput").ap()  # Wv.T
    cv_d = nc.dram_tensor("cv", [P, NS], F32, kind="ExternalInput").ap()
    bvb_d = nc.dram_tensor("bvb", [P, D], F32, kind="ExternalInput").ap()
    out_d = nc.dram_tensor("out", [S, D], F32, kind="ExternalOutput").ap()

    with tile.TileContext(nc) as tc:
        _emit(nc, tc, xi_d, xt_d, mt_d, wvt_d, cv_d, bvb_d, out_d)
    nc.compile()
    return nc


def _emit(nc, tc, xi_d, xt_d, mt_d, wvt_d, cv_d, bvb_d, out_d):
    NH = QC // P  # 4 q_tiles per chunk
    with (
        tc.tile_pool(name="const", bufs=1) as pc,
        tc.tile_pool(name="qkv", bufs=1) as pqkv,
        tc.tile_pool(name="psST", bufs=2, space="PSUM") as psST,
    ):
        # persistent activations
        kt = pqkv.tile([P, ND, S], BF16, name="kt", tag="kt")    # K~T[e,k]
        v = pqkv.tile([P, NS, D], BF16, name="v", tag="v")       # V[s,e]
        xi = pqkv.tile([P, ND, S], BF16, name="xi", tag="xi")    # XiT[e,q]

        # constants
        cv = pc.tile([P, NS], F32, name="cv", tag="cv")          # c per k-tile
        wl = pc.tile([1, P], BF16, name="wl", tag="wl")          # warmup lhsT
        wr = pc.tile([1, WC], BF16, name="wr", tag="wr")         # warmup rhs
        bv_bcast = pc.tile([P, D], F32, name="bv_bcast", tag="bv_bcast")
        ones_col = pc.tile([P, 1], BF16, name="ones_col", tag="ones_col")

        with (
            tc.tile_pool(name="w", bufs=1) as pw,
            tc.tile_pool(name="xs", bufs=3) as pxs,
            tc.tile_pool(name="psP", bufs=6, space="PSUM") as psP,
        ):
            mt_sb = pw.tile([P, ND, D], BF16, name="mt_sb", tag="mt_sb")
            wv_sb = pw.tile([P, ND, D], BF16, name="wv_sb", tag="wv_sb")

            nc.scalar.dma_start(cv[:], cv_d[:])
            nc.vector.memset(wl[:], 1.0)
            nc.vector.memset(wr[:], 1.0 / NWARM)

            # The start is latency-critical: chunk 0 is consumed d-outer
            # (see below), so interleave (mt_d, xc0_d) slice pairs across
            # both HWDGE rings in consumption order.  Everything else
            # rides SWDGE (gpsimd), which handles big 3D transfers well.
            wv_r = wvt_d.rearrange("(nd p) e -> p nd e", p=P)
            xi_r = xi_d.rearrange("(nd p) s -> p nd s", p=P)
            xt_r = xt_d.rearrange("(nd p) s -> p nd s", p=P)
            xc0 = pxs.tile([P, ND, QC], BF16, name="xc", tag="xs")
            for d in range(ND):
                eng = nc.sync if d % 2 == 0 else nc.scalar
                eng.dma_start(mt_sb[:, d, :], mt_d[d * P:(d + 1) * P, :])
                eng.dma_start(xc0[:, d, :], xt_d[d * P:(d + 1) * P, 0:QC])

            # --- warmup: bf16 matmuls accumulating 1/NWARM into what
            # becomes ones_col (live data, so not DCE-able).  Keeps PE
            # busy during the initial DMA wait and warms the HAM gate.
            ps_w = psP.tile([P, WC], F32, name="ps_w", tag="ps")
            for i in range(NWARM):
                nc.tensor.matmul(ps_w[:], wl[:], wr[:],
                                 start=(i == 0), stop=(i == NWARM - 1))
            nc.vector.tensor_copy(ones_col[:], ps_w[:, 0:1])

            # --- projections: stream XtT chunks once; K~ then V per chunk.
            # Order K0 K1 V0 K2 V1 K3 V2 V3 gives the wv DMA time to land.
            xcs = {}

            def load_chunk(c):
                xc = pxs.tile([P, ND, QC], BF16, name="xc", tag="xs")
                cs = slice(c * QC, (c + 1) * QC)
                nc.gpsimd.dma_start(xc[:], xt_r[:, :, cs])
                xcs[c] = xc

            xcs[0] = xc0
            load_chunk(1)
            nc.gpsimd.dma_start(wv_sb[:], wv_r[:])
            nc.gpsimd.dma_start(xi[:], xi_r[:])
            nc.gpsimd.dma_start(bv_bcast[:], bvb_d[:])

            def kproj0():
                # d-outer over all 8 PSUM banks: each d-pass needs only
                # the (mt_d, xc0_d) slice pair, so matmuls start as soon
                # as the first pair lands instead of after the full 3MB.
                pss = [psP.tile([P, QC], F32, name="ps", tag="ps")
                       for _ in range(6)]
                pss += [psST.tile([P, QC], F32, name="st_ps", tag="st")
                        for _ in range(2)]
                xc = xcs[0]
                for d in range(ND):
                    for et in range(ND):
                        nc.tensor.matmul(
                            pss[et][:], mt_sb[:, d, et * P:(et + 1) * P],
                            xc[:, d, :],
                            start=(d == 0), stop=(d == ND - 1))
                for et in range(ND):
                    nc.vector.tensor_copy(kt[:, et, 0:QC], pss[et][:])

            def kproj(c):
                xc = xcs[c]
                for et in range(ND):
                    ps = psP.tile([P, QC], F32, name="ps", tag="ps")
                    for d in range(ND):
                        nc.tensor.matmul(
                            ps[:], mt_sb[:, d, et * P:(et + 1) * P],
                            xc[:, d, :],
                            start=(d == 0), stop=(d == ND - 1))
                    nc.vector.tensor_copy(
                        kt[:, et, c * QC:(c + 1) * QC], ps[:])

            def vproj(c):
                xc = xcs[c]
                for si in range(NH):
                    st = c * NH + si
                    ps0 = psP.tile([P, EC], F32, name="ps0", tag="ps")
                    ps1 = psP.tile([P, EC], F32, name="ps1", tag="ps")
                    for d in range(ND):
                        lhs = xc[:, d, si * P:(si + 1) * P]
                        nc.tensor.matmul(ps0[:], lhs, wv_sb[:, d, 0:EC],
                                         start=(d == 0), stop=(d == ND - 1))
                        nc.tensor.matmul(ps1[:], lhs, wv_sb[:, d, EC:D],
                                         start=(d == 0), stop=(d == ND - 1))
                    nc.vector.tensor_copy(v[:, st, 0:EC], ps0[:])
                    nc.vector.tensor_copy(v[:, st, EC:D], ps1[:])

            kproj0()
            load_chunk(2)
            kproj(1)
            vproj(0)
            load_chunk(3)
            kproj(2)
            vproj(1)
            kproj(3)
            vproj(2)
            vproj(3)

        # --- attention ---
        with (
            tc.tile_pool(name="et", bufs=3) as pet,
            tc.tile_pool(name="outp", bufs=2) as pout,
            tc.tile_pool(name="stat", bufs=4) as pstat,
            tc.tile_pool(name="psAV", bufs=4, space="PSUM") as psAV,
            tc.tile_pool(name="psRS", bufs=2, space="PSUM") as psRS,
        ):
            for qc in range(NQC):
                # scores^T for this q chunk: ET[kk, q] = exp(scale*K~T.T@XiT + c)
                et_t = pet.tile([P, NS, QC], BF16, name="et_t", tag="et")
                for kk in range(NS):
                    st_ps = psST.tile([P, QC], F32, name="st_ps", tag="st")
                    for e in range(ND):
                        nc.tensor.matmul(
                            st_ps[:],
                            kt[:, e, kk * P:(kk + 1) * P],
                            xi[:, e, qc * QC:(qc + 1) * QC],
                            start=(e == 0), stop=(e == ND - 1))
                    nc.scalar.activation(et_t[:, kk, :], st_ps[:], AF.Exp,
                                         bias=cv[:, kk:kk + 1], scale=SCALE)

                # attended[q_t, :] = (ET.T @ V) * recip + bv
                for qs in range(NH):
                    a0 = psAV.tile([P, EC], F32, name="a0", tag="av")
                    a1 = psAV.tile([P, EC], F32, name="a1", tag="av")
                    rs = psRS.tile([P, 1], F32, name="rs", tag="rs")
                    for kk in range(NS):
                        lhs = et_t[:, kk, qs * P:(qs + 1) * P]
                        nc.tensor.matmul(rs[:], lhs, ones_col[:],
                                         start=(kk == 0), stop=(kk == NS - 1))
                        nc.tensor.matmul(a0[:], lhs, v[:, kk, 0:EC],
                                         start=(kk == 0), stop=(kk == NS - 1))
                        nc.tensor.matmul(a1[:], lhs, v[:, kk, EC:D],
                                         start=(kk == 0), stop=(kk == NS - 1))
                    recip = pstat.tile([P, 1], F32, name="recip", tag="recip")
                    nc.vector.reciprocal(recip[:], rs[:])
                    ob = pout.tile([P, D], F32, name="ob", tag="ob")
                    q_tile = qc * NH + qs
                    nc.vector.scalar_tensor_tensor(
                        ob[:, 0:EC], a0[:], recip[:], bv_bcast[:, 0:EC],
                        op0=ALU.mult, op1=ALU.add)
                    nc.sync.dma_start(
                        out_d[q_tile * P:(q_tile + 1) * P, 0:EC], ob[:, 0:EC])
                    nc.vector.scalar_tensor_tensor(
                        ob[:, EC:D], a1[:], recip[:], bv_bcast[:, EC:D],
                        op0=ALU.mult, op1=ALU.add)
                    nc.scalar.dma_start(
                        out_d[q_tile * P:(q_tile + 1) * P, EC:D], ob[:, EC:D])


def get_nc():
    if "nc" not in _CACHE:
        _CACHE["nc"] = _build_nc()
    return _CACHE["nc"]


def _prep_inputs(image_emb, text_emb, Wq, bq, Wk, bk, Wv, bv):
    bf = ml_dtypes.bfloat16
    xi = np.asarray(image_emb).astype(bf)   # [B, S, D]
    xt = np.asarray(text_emb).astype(bf)
    xiT = np.ascontiguousarray(xi.transpose(0, 2, 1))  # [B, D, S]
    xtT = np.ascontiguousarray(xt.transpose(0, 2, 1))
    Wq32 = np.asarray(Wq, dtype=np.float32)
    Wk32 = np.asarray(Wk, dtype=np.float32)
    mt = np.ascontiguousarray(Wk32.T @ Wq32).astype(bf)       # M^T = Wk^T Wq
    wvt = np.ascontiguousarray(np.asarray(Wv).T).astype(bf)
    bq32 = np.asarray(bq, dtype=np.float32)
    # per-k score bias that survives softmax: c = Xt @ (Wk^T bq)
    u = Wk32.T @ bq32                                          # [D]
    c = np.asarray(text_emb, dtype=np.float32) @ u             # [B, S]
    cv = np.ascontiguousarray(c.reshape(B, NS, P).transpose(0, 2, 1))
    bvb = np.ascontiguousarray(
        np.broadcast_to(np.asarray(bv, dtype=np.float32), (P, D)))
    in_maps = []
    for b in range(B):
        in_maps.append({
            "xiT": xiT[b], "xtT": xtT[b],
            "mt": mt, "wvt": wvt,
            "cv": cv[b], "bvb": bvb,
        })
    return in_maps


def run(image_emb, text_emb, Wq, bq, Wk, bk, Wv, bv, trace=False, **spmd_kwargs):
    nc = get_nc()
    in_maps = _prep_inputs(image_emb, text_emb, Wq, bq, Wk, bk, Wv, bv)
    res = run_bass_kernel_spmd(nc, in_maps, list(range(B)), trace=trace,
                               **spmd_kwargs)
    out = np.stack([res.results[b]["out"] for b in range(B)], axis=0)
    return out, res


def kernel(image_emb, text_emb, edge_index=None, Wq=None, bq=None, Wk=None,
           bk=None, Wv=None, bv=None, **_unused):
    out, _ = run(image_emb, text_emb, Wq, bq, Wk, bk, Wv, bv, trace=False)
    return out


# revision 25
# speedup vs baseline: 1.0554x; 1.0333x over previous
"""Trainium2 Bass kernel for single-head cross-modal attention.

Problem: B=8, S=2048, D=1024 (fp32 inputs)
    q = image_emb @ Wq.T + bq
    k = text_emb  @ Wk.T + bk
    v = text_emb  @ Wv.T + bv
    out = softmax(q @ k.T / sqrt(D)) @ v

Sharding: data-parallel over batch — core b handles batch element b.

Key algebraic trick: scores = (Xi Wq^T + bq)(Xt Wk^T + bk)^T.  The
per-q-row bias terms cancel in softmax, so
    softmax(scores) = softmax(Xi M Xt^T + 1 c^T),   M  = Wq^T Wk
                                                    c  = Xt (Wk^T bq)
M and c are precomputed on host (fp32), so the on-chip Q projection
disappears entirely: K~^T = M Xt^T takes the place of both projections
on the scores path (saves 2.1e9 MACs/core, ~14%).

Per-core dataflow (all matmuls bf16 with fp32 PSUM accumulation):
  - X^T prepared on host (bf16 cast + transpose), streamed in 512-col
    chunks; each chunk feeds BOTH the K~ projection and the V
    projection (XtT is DMA'd once).
  - K~T in [e, s] layout, V in natural [s, e] layout.
  - scores^T [k_part, q_free] = K~T_tile.T @ XiT, so exp(scores)^T is
    directly the stationary operand of the P@V matmul: the 2048x2048
    probability matrix is never transposed on chip.
  - softmax without max-subtraction (scores ~ N(0,1), |s| <= ~6: exp is
    safe in fp32).  The c bias rides the exp activation's per-partition
    bias operand.  Row sums come from N=1 matmuls against a ones
    column (sharing the stationary operand with the P@V matmuls).
  - final normalize fused: out = (att_psum * recip) + bv_bcast in one
    DVE op per 512-wide chunk; output DMA split across sync+scalar
    rings.
  - a short warmup matmul stream on memset data bridges the initial
    DMA wait and lifts the PE HAM clock gate before real work arrives
    (the warmup accumulates zeros into what becomes the bv broadcast,
    so it cannot be dead-code-eliminated).
"""

import sys
import os

for _p in ("/opt/trn_rl_repo", "/root/.axon_site/_ro/trn_rl_repo"):
    if os.path.isdir(_p) and _p not in sys.path:
        sys.path.insert(0, _p)

import numpy as np
import ml_dtypes

import concourse.bass as bass
import concourse.mybir as mybir
import concourse.tile as tile
from concourse import bacc
from concourse.bass_utils import run_bass_kernel_spmd

BF16 = mybir.dt.bfloat16
F32 = mybir.dt.float32
AF = mybir.ActivationFunctionType
ALU = mybir.AluOpType

B, S, D = 8, 2048, 1024
P = 128
ND = D // P          # 8  d/e tiles
NS = S // P          # 16 s tiles
QC = 512             # chunk width (matmul free dim / PSUM bank)
NQC = S // QC        # 4
EC = 512             # e chunk width for V / output
SCALE = 1.0 / float(np.sqrt(D))
NWARM = 16           # warmup matmuls (HAM + DMA-wait bridge)
WC = 256             # warmup matmul free dim

_CACHE = {}


def _build_nc():
    nc = bacc.Bacc("TRN2", target_bir_lowering=False, debug=False, num_devices=8)

    xi_d = nc.dram_tensor("xiT", [D, S], BF16, kind="ExternalInput").ap()
    xt_d = nc.dram_tensor("xtT", [D, S], BF16, kind="ExternalInput").ap()
    mt_d = nc.dram_tensor("mt", [D, D], BF16, kind="ExternalInput").ap()   # Wk.T@Wq
    wvt_d = nc.dram_tensor("wvt", [D, D], BF16, kind="ExternalInput").ap()  # Wv.T
    cv_d = nc.dram_tensor("cv", [P, NS], F32, kind="ExternalInput").ap()
    bvb_d = nc.dram_tensor("bvb", [P, D], F32, kind="ExternalInput").ap()
    out_d = nc.dram_tensor("out", [S, D], F32, kind="ExternalOutput").ap()

    with tile.TileContext(nc) as tc:
        _emit(nc, tc, xi_d, xt_d, mt_d, wvt_d, cv_d, bvb_d, out_d)
    nc.compile()
    return nc


def _emit(nc, tc, xi_d, xt_d, mt_d, wvt_d, cv_d, bvb_d, out_d):
    NH = QC // P  # 4 q_tiles per chunk
    with (
        tc.tile_pool(name="const", bufs=1) as pc,
        tc.tile_pool(name="qkv", bufs=1) as pqkv,
        tc.tile_pool(name="psST", bufs=2, space="PSUM") as psST,
    ):
        # persistent activations
        kt = pqkv.tile([P, ND, S], BF16, name="kt", tag="kt")    # K~T[e,k]
        v = pqkv.tile([P, NS, D], BF16, name="v", tag="v")       # V[s,e]
        xi = pqkv.tile([P, ND, S], BF16, name="xi", tag="xi")    # XiT[e,q]

        # constants
        cv = pc.tile([P, NS], F32, name="cv", tag="cv")          # c per k-tile
        wl = pc.tile([1, P], BF16, name="wl", tag="wl")          # warmup lhsT
        wr = pc.tile([1, WC], BF16, name="wr", tag="wr")         # warmup rhs
        bv_bcast = pc.tile([P, D], F32, name="bv_bcast", tag="bv_bcast")
        ones_col = pc.tile([P, 1], BF16, name="ones_col", tag="ones_col")

        with (
            tc.tile_pool(name="w", bufs=1) as pw,
            tc.tile_pool(name="xs", bufs=3) as pxs,
            tc.tile_pool(name="psP", bufs=6, space="PSUM") as psP,
        ):
            mt_sb = pw.tile([P, ND, D], BF16, name="mt_sb", tag="mt_sb")
            wv_sb = pw.tile([P, ND, D], BF16, name="wv_sb", tag="wv_sb")

            nc.scalar.dma_start(cv[:], cv_d[:])
            nc.vector.memset(wl[:], 1.0)
            nc.vector.memset(wr[:], 1.0 / NWARM)

            # The start is latency-critical: chunk 0 is consumed d-outer
            # (see below) at ~1.7us per (mt_d, xc0_d) pair.  SWDGE
            # (gpsimd) pipelines small transfers at ~1.4us each, HWDGE
            # rings serialize at ~2.3us each — so mt streams per-d on
            # gpsimd while xc0 slices alternate across both HWDGE rings.
            wv_r = wvt_d.rearrange("(nd p) e -> p nd e", p=P)
            xi_r = xi_d.rearrange("(nd p) s -> p nd s", p=P)
            xt_r = xt_d.rearrange("(nd p) s -> p nd s", p=P)
            xc0 = pxs.tile([P, ND, QC], BF16, name="xc", tag="xs")
            for d in range(ND):
                nc.gpsimd.dma_start(mt_sb[:, d, :], mt_d[d * P:(d + 1) * P, :])
                eng = nc.sync if d % 2 == 0 else nc.scalar
                eng.dma_start(xc0[:, d, :], xt_d[d * P:(d + 1) * P, 0:QC])

            # --- warmup: bf16 matmuls accumulating 1/NWARM into what
            # becomes ones_col (live data, so not DCE-able).  Keeps PE
            # busy during the initial DMA wait and warms the HAM gate.
            ps_w = psP.tile([P, WC], F32, name="ps_w", tag="ps")
            for i in range(NWARM):
                nc.tensor.matmul(ps_w[:], wl[:], wr[:],
                                 start=(i == 0), stop=(i == NWARM - 1))
            nc.vector.tensor_copy(ones_col[:], ps_w[:, 0:1])

            # --- projections: stream XtT chunks once; K~ then V per chunk.
            # Order K0 K1 V0 K2 V1 K3 V2 V3 gives the wv DMA time to land.
            xcs = {}

            def load_chunk(c, hwdge=False):
                xc = pxs.tile([P, ND, QC], BF16, name="xc", tag="xs")
                cs = slice(c * QC, (c + 1) * QC)
                if hwdge:
                    for d in range(ND):
                        eng = nc.sync if d % 2 == 0 else nc.scalar
                        eng.dma_start(
                            xc[:, d, :],
                            xt_d[d * P:(d + 1) * P, c * QC:(c + 1) * QC])
                else:
                    nc.gpsimd.dma_start(xc[:], xt_r[:, :, cs])
                xcs[c] = xc

            xcs[0] = xc0
            load_chunk(1)
            nc.gpsimd.dma_start(wv_sb[:], wv_r[:])
            load_chunk(2, hwdge=True)
            nc.gpsimd.dma_start(xi[:], xi_r[:])
            nc.gpsimd.dma_start(bv_bcast[:], bvb_d[:])

            def kproj0():
                # d-outer over all 8 PSUM banks: each d-pass needs only
                # the (mt_d, xc0_d) slice pair, so matmuls start as soon
                # as the first pair lands instead of after the full 3MB.
                pss = [psP.tile([P, QC], F32, name="ps", tag="ps")
                       for _ in range(6)]
                pss += [psST.tile([P, QC], F32, name="st_ps", tag="st")
                        for _ in range(2)]
                xc = xcs[0]
                for d in range(ND):
                    for et in range(ND):
                        nc.tensor.matmul(
                            pss[et][:], mt_sb[:, d, et * P:(et + 1) * P],
                            xc[:, d, :],
                            start=(d == 0), stop=(d == ND - 1))
                for et in range(ND):
                    nc.vector.tensor_copy(kt[:, et, 0:QC], pss[et][:])

            def kproj(c):
                xc = xcs[c]
                for et in range(ND):
                    ps = psP.tile([P, QC], F32, name="ps", tag="ps")
                    for d in range(ND):
                        nc.tensor.matmul(
                            ps[:], mt_sb[:, d, et * P:(et + 1) * P],
                            xc[:, d, :],
                            start=(d == 0), stop=(d == ND - 1))
                    nc.vector.tensor_copy(
                        kt[:, et, c * QC:(c + 1) * QC], ps[:])

            def vproj(c):
                xc = xcs[c]
                for si in range(NH):
                    st = c * NH + si
                    ps0 = psP.tile([P, EC], F32, name="ps0", tag="ps")
                    ps1 = psP.tile([P, EC], F32, name="ps1", tag="ps")
                    for d in range(ND):
                        lhs = xc[:, d, si * P:(si + 1) * P]
                        nc.tensor.matmul(ps0[:], lhs, wv_sb[:, d, 0:EC],
                                         start=(d == 0), stop=(d == ND - 1))
                        nc.tensor.matmul(ps1[:], lhs, wv_sb[:, d, EC:D],
                                         start=(d == 0), stop=(d == ND - 1))
                    nc.vector.tensor_copy(v[:, st, 0:EC], ps0[:])
                    nc.vector.tensor_copy(v[:, st, EC:D], ps1[:])

            kproj0()
            load_chunk(3)
            kproj(1)
            vproj(0)
            kproj(2)
            vproj(1)
            kproj(3)
            vproj(2)
            vproj(3)

        # --- attention ---
        with (
            tc.tile_pool(name="et", bufs=3) as pet,
            tc.tile_pool(name="outp", bufs=2) as pout,
            tc.tile_pool(name="stat", bufs=4) as pstat,
            tc.tile_pool(name="psAV", bufs=4, space="PSUM") as psAV,
            tc.tile_pool(name="psRS", bufs=2, space="PSUM") as psRS,
        ):
            for qc in range(NQC):
                # scores^T for this q chunk: ET[kk, q] = exp(scale*K~T.T@XiT + c)
                et_t = pet.tile([P, NS, QC], BF16, name="et_t", tag="et")
                for kk in range(NS):
                    st_ps = psST.tile([P, QC], F32, name="st_ps", tag="st")
                    for e in range(ND):
                        nc.tensor.matmul(
                            st_ps[:],
                            kt[:, e, kk * P:(kk + 1) * P],
                            xi[:, e, qc * QC:(qc + 1) * QC],
                            start=(e == 0), stop=(e == ND - 1))
                    nc.scalar.activation(et_t[:, kk, :], st_ps[:], AF.Exp,
                                         bias=cv[:, kk:kk + 1], scale=SCALE)

                # attended[q_t, :] = (ET.T @ V) * recip + bv
                for qs in range(NH):
                    a0 = psAV.tile([P, EC], F32, name="a0", tag="av")
                    a1 = psAV.tile([P, EC], F32, name="a1", tag="av")
                    rs = psRS.tile([P, 1], F32, name="rs", tag="rs")
                    for kk in range(NS):
                        lhs = et_t[:, kk, qs * P:(qs + 1) * P]
                        nc.tensor.matmul(rs[:], lhs, ones_col[:],
                                         start=(kk == 0), stop=(kk == NS - 1))
                        nc.tensor.matmul(a0[:], lhs, v[:, kk, 0:EC],
                                         start=(kk == 0), stop=(kk == NS - 1))
                        nc.tensor.matmul(a1[:], lhs, v[:, kk, EC:D],
                                         start=(kk == 0), stop=(kk == NS - 1))
                    recip = pstat.tile([P, 1], F32, name="recip", tag="recip")
                    nc.vector.reciprocal(recip[:], rs[:])
                    ob = pout.tile([P, D], F32, name="ob", tag="ob")
                    q_tile = qc * NH + qs
                    nc.vector.scalar_tensor_tensor(
                        ob[:, 0:EC], a0[:], recip[:], bv_bcast[:, 0:EC],
                        op0=ALU.mult, op1=ALU.add)
                    nc.sync.dma_start(
                        out_d[q_tile * P:(q_tile + 1) * P, 0:EC], ob[:, 0:EC])
                    nc.vector.scalar_tensor_tensor(
                        ob[:, EC:D], a1[:], recip[:], bv_bcast[:, EC:D],
                        op0=ALU.mult, op1=ALU.add)
                    nc.scalar.dma_start(
                        out_d[q_tile * P:(q_tile + 1) * P, EC:D], ob[:, EC:D])


def get_nc():
    if "nc" not in _CACHE:
        _CACHE["nc"] = _build_nc()
    return _CACHE["nc"]


def _prep_inputs(image_emb, text_emb, Wq, bq, Wk, bk, Wv, bv):
    bf = ml_dtypes.bfloat16
    xi = np.asarray(image_emb).astype(bf)   # [B, S, D]
    xt = np.asarray(text_emb).astype(bf)
    xiT = np.ascontiguousarray(xi.transpose(0, 2, 1))  # [B, D, S]
    xtT = np.ascontiguousarray(xt.transpose(0, 2, 1))
    Wq32 = np.asarray(Wq, dtype=np.float32)
    Wk32 = np.asarray(Wk, dtype=np.float32)
    mt = np.ascontiguousarray(Wk32.T @ Wq32).astype(bf)       # M^T = Wk^T Wq
    wvt = np.ascontiguousarray(np.asarray(Wv).T).astype(bf)
    bq32 = np.asarray(bq, dtype=np.float32)
    # per-k score bias that survives softmax: c = Xt @ (Wk^T bq)
    u = Wk32.T @ bq32                                          # [D]
    c = np.asarray(text_emb, dtype=np.float32) @ u             # [B, S]
    cv = np.ascontiguousarray(c.reshape(B, NS, P).transpose(0, 2, 1))
    bvb = np.ascontiguousarray(
        np.broadcast_to(np.asarray(bv, dtype=np.float32), (P, D)))
    in_maps = []
    for b in range(B):
        in_maps.append({
            "xiT": xiT[b], "xtT": xtT[b],
            "mt": mt, "wvt": wvt,
            "cv": cv[b], "bvb": bvb,
        })
    return in_maps


def run(image_emb, text_emb, Wq, bq, Wk, bk, Wv, bv, trace=False, **spmd_kwargs):
    nc = get_nc()
    in_maps = _prep_inputs(image_emb, text_emb, Wq, bq, Wk, bk, Wv, bv)
    res = run_bass_kernel_spmd(nc, in_maps, list(range(B)), trace=trace,
                               **spmd_kwargs)
    out = np.stack([res.results[b]["out"] for b in range(B)], axis=0)
    return out, res


def kernel(image_emb, text_emb, edge_index=None, Wq=None, bq=None, Wk=None,
           bk=None, Wv=None, bv=None, **_unused):
    out, _ = run(image_emb, text_emb, Wq, bq, Wk, bk, Wv, bv, trace=False)
    return out
